# revision 1
# baseline (speedup 1.0000x reference)
"""ConvCapsuleLayer Trainium2 kernel.

Strategy:
  - Data-parallel over batch B=16 across 8 cores (B_local=2 per core).
  - Conv (5x5, SAME, Ai=32 -> Co*Ao=256) done on the PE as x-stationary
    matmuls: lhsT = shifted x patches [K=(tap,ai), M=128 pixels],
    rhs = repacked W [K, 256], accumulated over 7 tap-groups in PSUM.
    Output lands directly in routing layout: [pixels(partitions), ao, co].
  - fp16 for votes and all big elementwise ops (DVE 2x mode), f32 for
    accumulations (reduces, preact, logits).
  - Dynamic routing (3 iters) per pixel entirely on-chip:
    softmax over co, preact = sum_ci route*votes + bias, squash,
    agreement = sum_ao votes*act. Squash's sqrt via exp(0.5*ln(x)) so the
    whole kernel uses one ACT table set (natural_log_exp).
  - Iteration-1 shortcut: route is uniform 1/16, so preact1 rides the conv
    via an extra "sum over ci" input plane (computed host-side).
"""

import os
import sys

import numpy as np

sys.path.insert(0, "/opt/trn_rl_repo")

from contextlib import ExitStack

import concourse.bacc as bacc
import concourse.bass as bass
import concourse.mybir as mybir
import concourse.tile as tile
from concourse.bass_utils import run_bass_kernel_spmd

F16 = mybir.dt.float16
F32 = mybir.dt.float32
AX = mybir.AxisListType
OP = mybir.AluOpType
AF = mybir.ActivationFunctionType

N_CORES = 8
B_FULL, H, Wd, Ci, Ai = 16, 32, 32, 8, 32
K, Co, Ao = 5, 16, 16
B_LOC = B_FULL // N_CORES  # 2
G = B_LOC * 8  # 16 row-quad groups per core
NQ = 4  # process routing in 4 quarters of 4 groups

_cache = {}


def _build_program():
    nc = bacc.Bacc(None, target_bir_lowering=False)
    xpad_d = nc.dram_tensor(
        "xpad", [B_LOC, Ci + 1, 2, 4, Ai, 36, 32], F16, kind="ExternalInput"
    )
    wstk_d = nc.dram_tensor("wstk", [7, 128, 256], F16, kind="ExternalInput")
    bias_d = nc.dram_tensor("biasin", [128, 2, 256], F32, kind="ExternalInput")
    out_d = nc.dram_tensor("out", [B_LOC, H, Wd, Co, Ao], F32, kind="ExternalOutput")

    with tile.TileContext(nc) as tc, ExitStack() as ctx:
        const_p = ctx.enter_context(tc.tile_pool(name="const", bufs=1))
        votes_p = ctx.enter_context(tc.tile_pool(name="votes", bufs=1))
        xrep_p = ctx.enter_context(tc.tile_pool(name="xrep", bufs=2))
        psum_p = ctx.enter_context(
            tc.tile_pool(name="psum", bufs=8, space=bass.MemorySpace.PSUM)
        )
        tmp_p = ctx.enter_context(tc.tile_pool(name="tmp", bufs=2))
        small_p = ctx.enter_context(tc.tile_pool(name="small", bufs=2))
        tiny_p = ctx.enter_context(tc.tile_pool(name="tiny", bufs=3))
        out_p = ctx.enter_context(tc.tile_pool(name="outs", bufs=2))

        # ---- constants ----
        # load W stacks: DRAM [7,128,256] -> SBUF [128, 7, 256]
        wstk = const_p.tile([128, 7, 256], F16)
        for s in range(7):
            nc.sync.dma_start(wstk[:, s], wstk_d[s])
        bias_rep = const_p.tile([128, 2, 256], F32)
        nc.sync.dma_start(bias_rep[:], bias_d[:])
        bias16 = bias_rep[:, 0]  # [128, 256] = 16*b in (ao, co) order
        bias1 = bias_rep[:, 1]  # [128, 256] = b in (ao, co) order
        bias1t = bias_rep[:, 1].rearrange("p (ao co) -> p ao co", ao=16)

        # votes tiles per quarter: [128, 4, Ci+1, 256] fp16
        votes_q = [
            votes_p.tile([128, 4, Ci + 1, 16, 16], F16, name=f"votes{q}")
            for q in range(NQ)
        ]

        # ---- conv ----
        # xrep: groups g=dx 0..3 hold the dx-shifted plane (row stride 32,
        # vertical pad 2+2); row shifts (dy) come from the flat AP offset.
        # xrep2: groups g=dy 0..3 of the dx=4-shifted plane.
        for bb in range(B_LOC):
            for ci in range(Ci + 1):
                xrep = xrep_p.tile([128, 36 * 32], F16, tag="xrepA")
                xrep2 = xrep_p.tile([128, 36 * 32], F16, tag="xrepB")
                nc.gpsimd.dma_start(
                    xrep[:],
                    xpad_d[bb, ci, 0].rearrange("s ai r c -> (s ai) (r c)"),
                )
                nc.gpsimd.dma_start(
                    xrep2[:],
                    xpad_d[bb, ci, 1].rearrange("s ai r c -> (s ai) (r c)"),
                )
                for yq in range(8):
                    ps = psum_p.tile([128, 256], F32, tag="convps")
                    for dy in range(5):
                        o = (4 * yq + dy) * 32
                        nc.tensor.matmul(
                            ps[:],
                            xrep[:, o : o + 128],
                            wstk[:, dy],
                            start=(dy == 0),
                            stop=False,
                        )
                    o = 4 * yq * 32
                    nc.tensor.matmul(
                        ps[:], xrep2[:, o : o + 128], wstk[:, 5], start=False, stop=False
                    )
                    o = (4 * yq + 4) * 32
                    nc.tensor.matmul(
                        ps[:],
                        xrep2[0:32, o : o + 128],
                        wstk[0:32, 6],
                        start=False,
                        stop=True,
                    )
                    q, gg = divmod(bb * 8 + yq, 4)
                    nc.scalar.copy(
                        votes_q[q][:, gg, ci].rearrange("p a c -> p (a c)"), ps[:]
                    )

        # ---- routing, per quarter ----
        inv16 = 1.0 / 16.0
        for q in range(NQ):
            vq = votes_q[q]
            V = vq[:, :, 0:Ci]  # [128, 4, ci, ao, co] fp16
            Vs = vq[:, :, Ci]  # [128, 4, ao, co] fp16

            logits = small_p.tile([128, 4, Ci, 16], F32, tag="logits")
            logits2 = small_p.tile([128, 4, Ci, 16], F32, tag="logits2")

            def squash_and_act(preactB, scale, act_out, last=False):
                # preactB: [128, 4, ao, co] f32 (raw; true preact = scale*raw)
                # act_out: fp16 [128,4,ao,co] (iters 1-2) or f32 (co,ao) (iter3)
                sq = tmp_p.tile([128, 4, 16, 16], F16, tag="sqact")
                nc.vector.tensor_tensor(sq[:], preactB[:], preactB[:], OP.mult)
                if last:
                    ns = tiny_p.tile([128, 4, 16], F32, tag="ns")
                    nc.vector.tensor_reduce(ns[:], sq[:], axis=AX.X, op=OP.add)
                else:
                    ns = tiny_p.tile([128, 4, 16], F32, tag="ns")
                    nc.vector.tensor_reduce(
                        ns[:], sq[:].transpose([0, 1, 3, 2]), axis=AX.X, op=OP.add
                    )
                lnv = tiny_p.tile([128, 4, 16], F32, tag="lnv")
                nc.scalar.activation(lnv[:], ns[:], AF.Ln, scale=scale * scale)
                sqr = tiny_p.tile([128, 4, 16], F32, tag="sqr")
                nc.scalar.activation(sqr[:], lnv[:], AF.Exp, scale=0.5)
                onep = tiny_p.tile([128, 4, 16], F32, tag="onep")
                nc.vector.tensor_scalar(
                    onep[:], ns[:], scale * scale, 1.0, op0=OP.mult, op1=OP.add
                )
                rec = tiny_p.tile([128, 4, 16], F32, tag="rec")
                nc.vector.reciprocal(rec[:], onep[:])
                fac = tiny_p.tile([128, 4, 16], F32, tag="fac")
                nc.vector.tensor_tensor(fac[:], sqr[:], rec[:], OP.mult)
                # act = raw*scale * sqrt(ns)/(1+ns) = raw * (scale*fac)
                if scale != 1.0:
                    nc.vector.tensor_scalar_mul(fac[:], fac[:], scale)
                fb = fac[:].unsqueeze(2).broadcast_to([128, 4, 16, 16])
                if last:
                    # preactB is [128,4,co,ao]; fac [128,4,co] bcast over ao
                    fb = fac[:].unsqueeze(3).broadcast_to([128, 4, 16, 16])
                nc.vector.tensor_tensor(act_out[:], preactB[:], fb, OP.mult)

            def agreement(act1, agr_out):
                # tmp2 = V * act (bcast ci) ; agr = sum_ao tmp2
                t2 = tmp_p.tile([128, 4, Ci, 16, 16], F16, tag="mbig", bufs=1)
                ab = act1[:].unsqueeze(2).broadcast_to([128, 4, Ci, 16, 16])
                nc.vector.tensor_tensor(t2[:], V, ab, OP.mult)
                nc.vector.tensor_reduce(
                    agr_out[:], t2[:].transpose([0, 1, 2, 4, 3]), axis=AX.X, op=OP.add
                )

            def softmax_route(lg, route_out):
                e = tmp_p.tile([128, 4, Ci, 16], F32, tag="expv", bufs=1)
                nc.scalar.activation(e[:], lg[:], AF.Exp)
                den = tiny_p.tile([128, 4, Ci], F32, tag="den")
                nc.vector.tensor_reduce(den[:], e[:], axis=AX.X, op=OP.add)
                rc = tiny_p.tile([128, 4, Ci], F32, tag="rc")
                nc.vector.reciprocal(rc[:], den[:])
                rb = rc[:].unsqueeze(3).broadcast_to([128, 4, Ci, 16])
                nc.vector.tensor_tensor(route_out[:], e[:], rb, OP.mult)

            def weighted_preact(route, preactB, bias_ap, transpose_out=False):
                t1 = tmp_p.tile([128, 4, Ci, 16, 16], F16, tag="mbig", bufs=1)
                rb = route[:].unsqueeze(3).broadcast_to([128, 4, Ci, 16, 16])
                nc.vector.tensor_tensor(t1[:], V, rb, OP.mult)
                pr = tmp_p.tile([128, 4, 16, 16], F32, tag="pr")
                nc.vector.tensor_reduce(
                    pr[:], t1[:].transpose([0, 1, 3, 4, 2]), axis=AX.X, op=OP.add
                )
                bb_ = bias_ap.unsqueeze(1).broadcast_to([128, 4, 16, 16])
                if transpose_out:
                    nc.vector.tensor_tensor(
                        preactB[:].transpose([0, 1, 3, 2]), pr[:], bb_, OP.add
                    )
                else:
                    nc.vector.tensor_tensor(preactB[:], pr[:], bb_, OP.add)

            # ---- iter 1 ----
            preactB = tmp_p.tile([128, 4, 16, 16], F32, tag="pB")
            b16 = bias16.rearrange("p (ao co) -> p ao co", ao=16)
            nc.vector.tensor_tensor(
                preactB[:], Vs, b16.unsqueeze(1).broadcast_to([128, 4, 16, 16]), OP.add
            )
            act1 = tmp_p.tile([128, 4, 16, 16], F16, tag="sqact")
            squash_and_act(preactB, inv16, act1)
            agreement(act1, logits)

            # ---- iter 2 ----
            route = small_p.tile([128, 4, Ci, 16], F16, tag="route")
            softmax_route(logits, route)
            preactB2 = tmp_p.tile([128, 4, 16, 16], F32, tag="pB")
            weighted_preact(route, preactB2, bias1.rearrange("p (ao co) -> p ao co", ao=16))
            act2 = tmp_p.tile([128, 4, 16, 16], F16, tag="sqact")
            squash_and_act(preactB2, 1.0, act2)
            agr2 = small_p.tile([128, 4, Ci, 16], F32, tag="agr2")
            agreement(act2, agr2)
            nc.vector.tensor_tensor(logits2[:], logits[:], agr2[:], OP.add)

            # ---- iter 3 ----
            route3 = small_p.tile([128, 4, Ci, 16], F16, tag="route")
            softmax_route(logits2, route3)
            preactB3 = tmp_p.tile([128, 4, 16, 16], F32, tag="pB")  # [*,*,co,ao]
            weighted_preact(route3, preactB3, bias1t, transpose_out=True)
            act_out = out_p.tile([128, 4, 16, 16], F32, tag="actout")
            squash_and_act(preactB3, 1.0, act_out, last=True)

            bb, half = divmod(q, 2)
            dst = out_d[bb, 16 * half : 16 * half + 16].rearrange(
                "(gg yy) x co ao -> (yy x) gg co ao", yy=4
            )
            nc.sync.dma_start(dst, act_out[:])

    nc.compile()
    return nc


def _prep_core_inputs(x_core, W, b):
    f16 = np.float16
    xr = np.transpose(x_core, (0, 3, 4, 1, 2)).astype(f16)  # [B_LOC, Ci, Ai, H, W]
    planes = np.zeros((B_LOC, Ci + 1, Ai, H, Wd), dtype=f16)
    planes[:, :Ci] = xr
    planes[:, Ci] = xr.astype(np.float32).sum(axis=1).astype(f16)
    # xpad[b, 0, s, ai, ci, r, c] = plane[r-2, c+s-2]   (s = dx shift 0..3)
    # xpad[b, 1, g, ai, ci, r, c] = plane[r+g-2, c+2]   (g = dy shift 0..3, dx=4)
    xpad = np.zeros((B_LOC, Ci + 1, 2, 4, Ai, 36, 32), dtype=f16)
    for s in range(4):
        c_lo = max(0, 2 - s)
        c_hi = min(32, 34 - s)
        xpad[:, :, 0, s, :, 2:34, c_lo:c_hi] = planes[
            :, :, :, :, c_lo + s - 2 : c_hi + s - 2
        ]
    for g in range(4):
        r_lo = max(0, 2 - g)
        r_hi = min(36, 34 - g)
        xpad[:, :, 1, g, :, r_lo:r_hi, 0:30] = planes[
            :, :, :, r_lo + g - 2 : r_hi + g - 2, 2:32
        ]
    # W stacks in (ao, co) output order:
    # slot dy (0..4): [(dx g, ai), 256]; slot 5: [(dy g, ai), 256] at dx=4;
    # slot 6: [ai, 256] for tap (4, 4).
    Wr = W.reshape(K, K, Ai, Co, Ao).transpose(0, 1, 2, 4, 3)  # [dy,dx,ai,ao,co]
    wstk = np.zeros((7, 128, 256), dtype=f16)
    for dy in range(5):
        wstk[dy] = (
            Wr[dy, 0:4].reshape(4 * Ai, Ao * Co).astype(f16)
        )  # [(dx,ai), (ao,co)]
    wstk[5] = Wr[0:4, 4].reshape(4 * Ai, Ao * Co).astype(f16)  # [(dy,ai), ...]
    wstk[6, :32] = Wr[4, 4].reshape(Ai, Ao * Co).astype(f16)
    bias_aoco = b[0, 0].T.reshape(256).astype(np.float32)  # (ao, co) order
    biasin = np.broadcast_to(
        np.stack([16.0 * bias_aoco, bias_aoco])[None], (128, 2, 256)
    ).copy()
    return {"xpad": xpad, "wstk": wstk, "biasin": biasin}


def kernel(x, W, b):
    if "nc" not in _cache:
        _cache["nc"] = _build_program()
    nc = _cache["nc"]
    in_maps = []
    for c in range(N_CORES):
        x_core = x[c * B_LOC : (c + 1) * B_LOC]
        in_maps.append(_prep_core_inputs(x_core, W, b))
    res = run_bass_kernel_spmd(nc, in_maps, list(range(N_CORES)))
    outs = [res.results[c]["out"] for c in range(N_CORES)]
    return np.concatenate(outs, axis=0).astype(np.float32)


if __name__ == "__main__":
    x = np.random.randn(16, 32, 32, 8, 32).astype(np.float32)
    W = np.random.randn(5, 5, 32, 256).astype(np.float32) * np.sqrt(2.0 / 800)
    b = np.full((1, 1, 16, 16), 0.1, dtype=np.float32)
    out = kernel(x, W, b)
    print(out.shape, out.dtype)



# revision 7
# speedup vs baseline: 1.7410x; 1.7410x over previous
"""ConvCapsuleLayer Trainium2 kernel (v2).

Strategy:
  - Data-parallel over batch B=16 across 8 cores (B_local=2 per core).
  - Conv (5x5, SAME, Ai=32 -> Co*Ao=256) done on the PE as x-stationary
    matmuls: lhsT = shifted x patches [K=(tap,ai), M=128 pixels],
    rhs = repacked W [K, 256], accumulated over 7 tap-groups in PSUM.
    Conv emitted in (bb, half) chunks so each routing quarter's votes
    complete at ~25/50/75/100% of the conv, keeping DVE fed early.
  - Votes layout [pix, g, ao, ci, co] fp16 (co innermost): every big DVE
    multiply hits 2x mode (stride-1 innermost on all operands) and all
    reductions (over ci for preact, over ao for agreement/norm) are
    contiguous tree-adds instead of strided tensor_reduces.
  - Dynamic routing (3 iters) per pixel on-chip: softmax over co (f32
    logits/exp for range safety), preact = sum_ci route*votes + bias,
    squash factor per (pixel, co), agreement = sum_ao votes*preact with
    the squash factor folded in afterwards (no act materialization in
    iters 1-2). sqrt via exp(0.5*ln(x)).
  - Iteration-1 shortcut: route is uniform 1/16, so preact1 rides the conv
    via an extra "sum over ci" input plane (computed host-side).
  - PSUM->SBUF vote copies alternate between ACT and Pool engines.
  - Output fp16 in (ao, co) order; host transposes to (co, ao) and casts.
"""

import os
import sys

import numpy as np

sys.path.insert(0, "/opt/trn_rl_repo")

from contextlib import ExitStack

import concourse.bacc as bacc
import concourse.bass as bass
import concourse.mybir as mybir
import concourse.tile as tile
from concourse.bass_utils import run_bass_kernel_spmd

F16 = mybir.dt.float16
F32 = mybir.dt.float32
AX = mybir.AxisListType
OP = mybir.AluOpType
AF = mybir.ActivationFunctionType

N_CORES = 8
B_FULL, H, Wd, Ci, Ai = 16, 32, 32, 8, 32
K, Co, Ao = 5, 16, 16
B_LOC = B_FULL // N_CORES  # 2
NQ = 4  # 4 routing quarters of 4 row-quad groups each

_cache = {}


def _build_program():
    nc = bacc.Bacc(None, target_bir_lowering=False)
    xpad_d = nc.dram_tensor(
        "xpad", [B_LOC, Ci + 1, 2, 4, Ai, 36, 32], F16, kind="ExternalInput"
    )
    wstk_d = nc.dram_tensor("wstk", [7, 128, 256], F16, kind="ExternalInput")
    bias_d = nc.dram_tensor("biasin", [128, 2, 256], F32, kind="ExternalInput")
    out_d = nc.dram_tensor("out", [B_LOC, H, Wd, Ao, Co], F16, kind="ExternalOutput")

    with tile.TileContext(nc) as tc, ExitStack() as ctx:
        const_p = ctx.enter_context(tc.tile_pool(name="const", bufs=1))
        votes_p = ctx.enter_context(tc.tile_pool(name="votes", bufs=1))
        xrep_p = ctx.enter_context(tc.tile_pool(name="xrep", bufs=3))
        psum_p = ctx.enter_context(
            tc.tile_pool(name="psum", bufs=8, space=bass.MemorySpace.PSUM)
        )
        big_p = ctx.enter_context(tc.tile_pool(name="big", bufs=2))
        tree_p = ctx.enter_context(tc.tile_pool(name="tree", bufs=1))
        mid_p = ctx.enter_context(tc.tile_pool(name="mid", bufs=2))
        tiny_p = ctx.enter_context(tc.tile_pool(name="tiny", bufs=2))
        out_p = ctx.enter_context(tc.tile_pool(name="outs", bufs=2))

        # ---- constants ----
        wstk = const_p.tile([128, 7, 256], F16)
        for s in range(7):
            nc.sync.dma_start(wstk[:, s], wstk_d[s])
        bias_rep = const_p.tile([128, 2, 256], F32)
        nc.sync.dma_start(bias_rep[:], bias_d[:])
        # (ao, co) order; bias16 = 16*b (iter-1 raw preact), bias1 = b
        bias_h = const_p.tile([128, 2, 256], F16)
        nc.scalar.copy(bias_h[:], bias_rep[:])
        bias16 = bias_h[:, 0].rearrange("p (ao co) -> p ao co", ao=16)
        bias1 = bias_h[:, 1].rearrange("p (ao co) -> p ao co", ao=16)

        # votes per quarter: [pix, g, ao, ci, co] fp16 + ci-sum plane
        votes_q = [
            votes_p.tile([128, 4, 16, Ci, 16], F16, name=f"votes{q}")
            for q in range(NQ)
        ]
        vsum_q = [
            votes_p.tile([128, 4, 16, 16], F16, name=f"vsum{q}") for q in range(NQ)
        ]

        ncopy = [0]

        def conv_quarter(bb, hf):
            """Conv for output rows 16*hf..16*hf+15 of batch bb -> quarter q."""
            q = 2 * bb + hf
            r0 = 16 * hf  # first padded row needed (rows r0..r0+19)
            for ci in range(Ci + 1):
                xrep = xrep_p.tile([128, 20 * 32], F16, tag="xrepA")
                xrep2 = xrep_p.tile([128, 20 * 32], F16, tag="xrepB")
                src = xpad_d[bb, ci, 0].rearrange("s ai r c -> (s ai) (r c)")
                nc.sync.dma_start(xrep[:], src[:, r0 * 32 : (r0 + 20) * 32])
                src2 = xpad_d[bb, ci, 1].rearrange("s ai r c -> (s ai) (r c)")
                nc.sync.dma_start(xrep2[:], src2[:, r0 * 32 : (r0 + 20) * 32])
                for yq in range(4):
                    ps = psum_p.tile([128, 256], F32, tag="convps")
                    for dy in range(5):
                        o = (4 * yq + dy) * 32
                        nc.tensor.matmul(
                            ps[:],
                            xrep[:, o : o + 128],
                            wstk[:, dy],
                            start=(dy == 0),
                            stop=False,
                        )
                    o = 4 * yq * 32
                    nc.tensor.matmul(
                        ps[:], xrep2[:, o : o + 128], wstk[:, 5], start=False,
                        stop=False,
                    )
                    o = (4 * yq + 4) * 32
                    nc.tensor.matmul(
                        ps[:],
                        xrep2[0:32, o : o + 128],
                        wstk[0:32, 6],
                        start=False,
                        stop=True,
                    )
                    if ci < Ci:
                        dst = votes_q[q][:, yq, :, ci, :]
                    else:
                        dst = vsum_q[q][:, yq]
                    nc.scalar.copy(dst, ps[:])
                    ncopy[0] += 1

        def squash_factor(ns, scale):
            """fac[g, co] = sqrt(s2*ns)/(1+s2*ns), from ns fp16 [128,4,16]."""
            s2 = scale * scale
            lnv = tiny_p.tile([128, 4, 16], F32, tag="lnv")
            nc.scalar.activation(lnv[:], ns[:], AF.Ln, scale=s2)
            sqr = tiny_p.tile([128, 4, 16], F32, tag="sqr")
            nc.scalar.activation(sqr[:], lnv[:], AF.Exp, scale=0.5)
            onep = tiny_p.tile([128, 4, 16], F32, tag="onep")
            nc.vector.tensor_scalar(onep[:], ns[:], s2, 1.0, op0=OP.mult, op1=OP.add)
            rec = tiny_p.tile([128, 4, 16], F32, tag="rec")
            nc.vector.reciprocal(rec[:], onep[:])
            fac = tiny_p.tile([128, 4, 16], F32, tag="fac")
            nc.vector.tensor_tensor(fac[:], sqr[:], rec[:], OP.mult)
            return fac

        def ns_tree(sq):
            """ns[g, co] = sum_ao sq[g, ao, co], contiguous tree adds."""
            n1 = tiny_p.tile([128, 4, 8, 16], F16, tag="ns1")
            nc.vector.tensor_tensor(n1[:], sq[:, :, 0:8], sq[:, :, 8:16], OP.add)
            n2 = tiny_p.tile([128, 4, 4, 16], F16, tag="ns2")
            nc.vector.tensor_tensor(n2[:], n1[:, :, 0:4], n1[:, :, 4:8], OP.add)
            n3 = tiny_p.tile([128, 4, 2, 16], F16, tag="ns3")
            nc.vector.tensor_tensor(n3[:], n2[:, :, 0:2], n2[:, :, 2:4], OP.add)
            ns = tiny_p.tile([128, 4, 16], F16, tag="ns")
            nc.vector.tensor_tensor(ns[:], n3[:, :, 0], n3[:, :, 1], OP.add)
            return ns

        def agreement_tree(V, pb):
            """agr0[g, ci, co] f32 = sum_ao V * pb (pb broadcast over ci)."""
            t2 = big_p.tile([128, 4, 16, Ci, 16], F16, tag="big")
            pbb = pb[:].unsqueeze(3).broadcast_to([128, 4, 16, Ci, 16])
            nc.vector.tensor_tensor(t2[:], V, pbb, OP.mult)
            a1 = tree_p.tile([128, 4, 8, Ci, 16], F16, tag="atr1")
            nc.vector.tensor_tensor(a1[:], t2[:, :, 0:8], t2[:, :, 8:16], OP.add)
            a2 = tree_p.tile([128, 4, 4, Ci, 16], F16, tag="atr2")
            nc.vector.tensor_tensor(a2[:], a1[:, :, 0:4], a1[:, :, 4:8], OP.add)
            a3 = tree_p.tile([128, 4, 2, Ci, 16], F16, tag="atr3")
            nc.vector.tensor_tensor(a3[:], a2[:, :, 0:2], a2[:, :, 2:4], OP.add)
            agr0 = mid_p.tile([128, 4, Ci, 16], F32, tag="agr0")
            nc.vector.tensor_tensor(agr0[:], a3[:, :, 0], a3[:, :, 1], OP.add)
            return agr0

        def weighted_preact(V, r, bias_ap):
            """pb[g, ao, co] fp16 = sum_ci V * r (r broadcast over ao) + bias."""
            t1 = big_p.tile([128, 4, 16, Ci, 16], F16, tag="big")
            rb = r[:].unsqueeze(2).broadcast_to([128, 4, 16, Ci, 16])
            nc.vector.tensor_tensor(t1[:], V, rb, OP.mult)
            p1 = tree_p.tile([128, 4, 16, 4, 16], F16, tag="ptr1")
            nc.vector.tensor_tensor(
                p1[:], t1[:, :, :, 0:4], t1[:, :, :, 4:8], OP.add
            )
            p2 = tree_p.tile([128, 4, 16, 2, 16], F16, tag="ptr2")
            nc.vector.tensor_tensor(
                p2[:], p1[:, :, :, 0:2], p1[:, :, :, 2:4], OP.add
            )
            p3 = tree_p.tile([128, 4, 16, 16], F16, tag="ptr3")
            nc.vector.tensor_tensor(p3[:], p2[:, :, :, 0], p2[:, :, :, 1], OP.add)
            pb = mid_p.tile([128, 4, 16, 16], F16, tag="pb")
            bb_ = bias_ap.unsqueeze(1).broadcast_to([128, 4, 16, 16])
            nc.vector.tensor_tensor(pb[:], p3[:], bb_, OP.add)
            return pb

        def softmax_route(lg):
            """r[g, ci, co] fp16 = softmax over co of lg f32."""
            e = mid_p.tile([128, 4, Ci, 16], F32, tag="expv")
            nc.scalar.activation(e[:], lg[:], AF.Exp)
            den = tiny_p.tile([128, 4, Ci], F32, tag="den")
            nc.vector.tensor_reduce(den[:], e[:], axis=AX.X, op=OP.add)
            rc = tiny_p.tile([128, 4, Ci], F32, tag="rc")
            nc.vector.reciprocal(rc[:], den[:])
            r = mid_p.tile([128, 4, Ci, 16], F16, tag="route")
            rcb = rc[:].unsqueeze(3).broadcast_to([128, 4, Ci, 16])
            nc.vector.tensor_tensor(r[:], e[:], rcb, OP.mult)
            return r

        def routing_quarter(q):
            V = votes_q[q][:]
            inv16 = 1.0 / 16.0

            # ---- iter 1 (uniform route; raw preact = vsum + 16*bias) ----
            pb1 = mid_p.tile([128, 4, 16, 16], F16, tag="pb")
            b16 = bias16.unsqueeze(1).broadcast_to([128, 4, 16, 16])
            nc.vector.tensor_tensor(pb1[:], vsum_q[q][:], b16, OP.add)
            sq1 = mid_p.tile([128, 4, 16, 16], F16, tag="sq")
            nc.scalar.activation(sq1[:], pb1[:], AF.Square)
            ns1 = ns_tree(sq1)
            fac1 = squash_factor(ns1, inv16)
            agr1 = agreement_tree(V, pb1)
            # fold the iter-1 uniform-route scale into fac1 (tiny op)
            fac1s = tiny_p.tile([128, 4, 16], F32, tag="facs")
            nc.vector.tensor_scalar_mul(fac1s[:], fac1[:], inv16)
            logits1 = mid_p.tile([128, 4, Ci, 16], F32, tag="logits1")
            f1b = fac1s[:].unsqueeze(2).broadcast_to([128, 4, Ci, 16])
            nc.vector.tensor_tensor(logits1[:], agr1[:], f1b, OP.mult)

            # ---- iter 2 ----
            r2 = softmax_route(logits1)
            pb2 = weighted_preact(V, r2, bias1)
            sq2 = mid_p.tile([128, 4, 16, 16], F16, tag="sq")
            nc.scalar.activation(sq2[:], pb2[:], AF.Square)
            ns2 = ns_tree(sq2)
            fac2 = squash_factor(ns2, 1.0)
            agr2 = agreement_tree(V, pb2)
            upd = mid_p.tile([128, 4, Ci, 16], F32, tag="upd")
            f2b = fac2[:].unsqueeze(2).broadcast_to([128, 4, Ci, 16])
            nc.vector.tensor_tensor(upd[:], agr2[:], f2b, OP.mult)
            logits2 = mid_p.tile([128, 4, Ci, 16], F32, tag="logits2")
            nc.vector.tensor_tensor(logits2[:], logits1[:], upd[:], OP.add)

            # ---- iter 3 ----
            r3 = softmax_route(logits2)
            pb3 = weighted_preact(V, r3, bias1)
            sq3 = mid_p.tile([128, 4, 16, 16], F16, tag="sq")
            nc.scalar.activation(sq3[:], pb3[:], AF.Square)
            ns3 = ns_tree(sq3)
            fac3 = squash_factor(ns3, 1.0)
            act3 = out_p.tile([128, 4, 16, 16], F16, tag="actout")
            f3b = fac3[:].unsqueeze(2).broadcast_to([128, 4, 16, 16])
            nc.vector.tensor_tensor(act3[:], pb3[:], f3b, OP.mult)

            bb, half = divmod(q, 2)
            dst = out_d[bb, 16 * half : 16 * half + 16].rearrange(
                "(gg yy) x ao co -> (yy x) gg ao co", yy=4
            )
            # ACT queue, not sync: the sync queue carries the next quarter's
            # xrep loads and must not stall behind routing completion.
            nc.scalar.dma_start(dst, act3[:])

        # ---- pipeline: conv quarter then its routing ----
        for bb in range(B_LOC):
            for hf in range(2):
                conv_quarter(bb, hf)
                routing_quarter(2 * bb + hf)

    nc.compile()
    return nc


def _prep_core_inputs(x_core, W, b):
    f16 = np.float16
    xr = np.transpose(x_core, (0, 3, 4, 1, 2)).astype(f16)  # [B_LOC, Ci, Ai, H, W]
    planes = np.zeros((B_LOC, Ci + 1, Ai, H, Wd), dtype=f16)
    planes[:, :Ci] = xr
    planes[:, Ci] = xr.astype(np.float32).sum(axis=1).astype(f16)
    # xpad[b, 0, s, ai, ci, r, c] = plane[r-2, c+s-2]   (s = dx shift 0..3)
    # xpad[b, 1, g, ai, ci, r, c] = plane[r+g-2, c+2]   (g = dy shift 0..3, dx=4)
    xpad = np.zeros((B_LOC, Ci + 1, 2, 4, Ai, 36, 32), dtype=f16)
    for s in range(4):
        c_lo = max(0, 2 - s)
        c_hi = min(32, 34 - s)
        xpad[:, :, 0, s, :, 2:34, c_lo:c_hi] = planes[
            :, :, :, :, c_lo + s - 2 : c_hi + s - 2
        ]
    for g in range(4):
        r_lo = max(0, 2 - g)
        r_hi = min(36, 34 - g)
        xpad[:, :, 1, g, :, r_lo:r_hi, 0:30] = planes[
            :, :, :, r_lo + g - 2 : r_hi + g - 2, 2:32
        ]
    # W stacks in (ao, co) output order:
    # slot dy (0..4): [(dx g, ai), 256]; slot 5: [(dy g, ai), 256] at dx=4;
    # slot 6: [ai, 256] for tap (4, 4).
    Wr = W.reshape(K, K, Ai, Co, Ao).transpose(0, 1, 2, 4, 3)  # [dy,dx,ai,ao,co]
    wstk = np.zeros((7, 128, 256), dtype=f16)
    for dy in range(5):
        wstk[dy] = (
            Wr[dy, 0:4].reshape(4 * Ai, Ao * Co).astype(f16)
        )  # [(dx,ai), (ao,co)]
    wstk[5] = Wr[0:4, 4].reshape(4 * Ai, Ao * Co).astype(f16)  # [(dy,ai), ...]
    wstk[6, :32] = Wr[4, 4].reshape(Ai, Ao * Co).astype(f16)
    bias_aoco = b[0, 0].T.reshape(256).astype(np.float32)  # (ao, co) order
    biasin = np.broadcast_to(
        np.stack([16.0 * bias_aoco, bias_aoco])[None], (128, 2, 256)
    ).copy()
    return {"xpad": xpad, "wstk": wstk, "biasin": biasin}


def kernel(x, W, b):
    if "nc" not in _cache:
        _cache["nc"] = _build_program()
    nc = _cache["nc"]
    in_maps = []
    for c in range(N_CORES):
        x_core = x[c * B_LOC : (c + 1) * B_LOC]
        in_maps.append(_prep_core_inputs(x_core, W, b))
    res = run_bass_kernel_spmd(nc, in_maps, list(range(N_CORES)))
    outs = [res.results[c]["out"] for c in range(N_CORES)]
    full = np.concatenate(outs, axis=0)  # [B, H, W, Ao, Co] fp16
    return np.ascontiguousarray(full.transpose(0, 1, 2, 4, 3)).astype(np.float32)


if __name__ == "__main__":
    x = np.random.randn(16, 32, 32, 8, 32).astype(np.float32)
    W = np.random.randn(5, 5, 32, 256).astype(np.float32) * np.sqrt(2.0 / 800)
    b = np.full((1, 1, 16, 16), 0.1, dtype=np.float32)
    out = kernel(x, W, b)
    print(out.shape, out.dtype)
